# revision 1
# baseline (speedup 1.0000x reference)
"""DeepFM Trainium2 Bass kernel.

Strategy: batch-parallel across 8 NeuronCores, embedding tables replicated.
Host prep builds a combined table [F*V, 17] (16 emb2 cols + 1 emb1 col) so a
single indirect-DMA descriptor per (sample, field) fetches everything.

Per core (NS=2048 samples, 16 tiles of 128):
  - indirect gather of 39 rows x 17 f32 per sample into [128, 39*17] tiles
  - DVE: scale by Xv (broadcast over the 17 row elems)
  - ACT: Square+accum -> sum of squares per sample; Copy+accum -> first order
  - PE: transpose e2 cols (5 chunks of <=128), then matmul vs [W1|S1] to get
    h1 (MLP layer 1, bias via ones-row trick) and s_e = sum_f e2 per e
  - DVE: 0.5*sum_e s_e^2 via tensor_tensor_reduce (init = global bias)
  - PE: h1^T transpose, h2 matmul (bias via ones-row), relu, row-sum
  - combine -> [128,1] totals, collected into [128,16], one final transpose
    and a single DMA of the [2048] shard.
"""

import os
from contextlib import nullcontext

import numpy as np

import concourse.bass as bass
import concourse.mybir as mybir
import concourse.tile as tile
from concourse import bacc
from concourse.bass import IndirectOffsetOnAxis
from concourse.bass_utils import run_bass_kernel_spmd
from concourse.masks import make_identity

F32 = mybir.dt.float32
I32 = mybir.dt.int32

F = 39
V = 100000
E = 16
ROW = E + 1          # 17: emb2 row + emb1 scalar
B = 16384
H1 = 32
H2 = 32
NCORES = 8
P = 128

# e2 transpose chunking: 128 cols = 8 fields x 16 elems
FPC = 8                                  # fields per chunk
NCHUNK = (F + FPC - 1) // FPC            # 5 chunks (last has 7 fields)


def _chunk_fields(c):
    f0 = c * FPC
    return f0, min(FPC, F - f0)


def build_nc(ns, v, group, num_devices, nq=1, repeat=1):
    """Build the per-core Bass program. ns = samples per core.

    repeat>1 wraps the whole body in a hardware loop (timing runs only).
    """
    nt = ns // P                  # tiles per core
    ng = nt // group              # gather groups
    assert nt % group == 0

    nc = bacc.Bacc(
        "TRN2", target_bir_lowering=False, debug=False, num_devices=num_devices,
        num_swdge_queues=nq,
    )

    tbl = nc.dram_tensor("tbl", [F * v, ROW], F32, kind="ExternalInput").ap()
    off = nc.dram_tensor("off", [ns, F], I32, kind="ExternalInput").ap()
    xv = nc.dram_tensor("xv", [ns, F], F32, kind="ExternalInput").ap()
    # rmat rows 0..623: [W1 | S1] (S1 = tiled eye(16)); row 624: [b1 | 0]
    rmat = nc.dram_tensor("rmat", [NCHUNK * P, H1 + E], F32, kind="ExternalInput").ap()
    # w2b rows 0..31: W2; row 32: b2
    w2b = nc.dram_tensor("w2b", [H1 + 1, H2], F32, kind="ExternalInput").ap()
    cbc = nc.dram_tensor("cbc", [P, 1], F32, kind="ExternalInput").ap()
    out = nc.dram_tensor("out", [ns], F32, kind="ExternalOutput").ap()

    NR = H1 + E  # 48 combined rhs cols

    with tile.TileContext(nc) as tc:
        with (
            tc.tile_pool(name="const", bufs=1) as cpool,
            tc.tile_pool(name="sb", bufs=2) as sb,
            tc.tile_pool(name="sc", bufs=2) as sc,
            tc.tile_pool(name="ps", bufs=2, space="PSUM") as ps,   # o48: 2 banks
            tc.tile_pool(name="ps1", bufs=1, space="PSUM") as ps1,  # tp(2) tp2 h2p resT
        ):
            ident = cpool.tile([P, P], F32)
            make_identity(nc, ident[:])

            # constants
            r_sb = cpool.tile([P, NCHUNK * NR], F32)   # chunk c at cols c*NR
            nc.sync.dma_start(
                out=r_sb[:].rearrange("p (c r) -> p c r", c=NCHUNK),
                in_=rmat.rearrange("(c p) r -> p c r", p=P),
            )
            w2_sb = cpool.tile([H1 + 1, H2], F32)
            nc.sync.dma_start(out=w2_sb[:], in_=w2b)
            cbc_sb = cpool.tile([P, 1], F32)
            nc.sync.dma_start(out=cbc_sb[:], in_=cbc)

            # manual double-buffered scT with persistent ones-row (bias trick)
            scT_bufs = []
            for i in range(2):
                t = cpool.tile([P, NCHUNK * P], F32, tag=f"scT{i}")
                # ones row for the last chunk lives at row 112; engines can only
                # address partition starts 0/32/64/96, so set rows 96..127 --
                # rows 96..111 are overwritten by every tile's PSUM copy.
                nc.vector.memset(t[96:P, (NCHUNK - 1) * P : NCHUNK * P], 1.0)
                scT_bufs.append(t)
            h1T_bufs = []
            for i in range(2):
                t = cpool.tile([H1 + 1, P], F32, tag=f"h1T{i}")
                nc.vector.memset(t[H1 : H1 + 1, :], 1.0)
                h1T_bufs.append(t)

            res = cpool.tile([P, nt], F32)

            gf = group * F
            loop_cm = tc.For_i(0, repeat, 1) if repeat > 1 else nullcontext()
            with loop_cm:
                for g in range(ng):
                    offt = sb.tile([P, gf], I32, tag="offt")
                    xvt = sb.tile([P, gf], F32, tag="xvt")
                    # dram [group*P, F] -> sbuf [p, (j, f)] with sample = j*P + p
                    nc.sync.dma_start(
                        out=offt[:].rearrange("p (j f) -> p j f", j=group),
                        in_=off[g * group * P : (g + 1) * group * P, :].rearrange(
                            "(j p) f -> p j f", p=P
                        ),
                    )
                    nc.sync.dma_start(
                        out=xvt[:].rearrange("p (j f) -> p j f", j=group),
                        in_=xv[g * group * P : (g + 1) * group * P, :].rearrange(
                            "(j p) f -> p j f", p=P
                        ),
                    )
                    raw = sb.tile([P, gf * ROW], F32, tag="raw")
                    # HW indirect DMA consumes ONE offset per dest partition, so
                    # each (tile, field) needs its own 128-row gather instruction.
                    for kk in range(gf):
                        inst = nc.gpsimd.indirect_dma_start(
                            out=raw[:, kk * ROW : (kk + 1) * ROW],
                            out_offset=None,
                            in_=tbl,
                            in_offset=IndirectOffsetOnAxis(
                                ap=offt[:, kk : kk + 1], axis=0
                            ),
                        )
                        if nq > 1 and kk % nq != 0:
                            inst.ins.queue = f"qPoolDynamic{kk % nq}"
                    raw3 = raw[:].rearrange("p (k r) -> p k r", r=ROW)

                    # scale by Xv; write e2 cols contiguous (HW matmul operands
                    # must be single-free-dim APs) and emb1 col separately
                    scaledE = sb.tile([P, gf * E], F32, tag="scaledE")
                    nc.vector.tensor_tensor(
                        out=scaledE[:].rearrange("p (k e) -> p k e", e=E),
                        in0=raw3[:, :, 0:E],
                        in1=xvt[:].to_broadcast([P, gf, E]),
                        op=mybir.AluOpType.mult,
                    )
                    scaledF = sb.tile([P, gf], F32, tag="scaledF")
                    nc.vector.tensor_tensor(
                        out=scaledF[:],
                        in0=raw3[:, :, E].squeeze(),
                        in1=xvt[:],
                        op=mybir.AluOpType.mult,
                    )

                    for j in range(group):
                        t = g * group + j

                        # ACT: sum of squares of e2 cols, first-order sum
                        sq_scr = sc.tile([P, F * E], F32, tag="sq_scr")
                        sqtot = sc.tile([P, 1], F32, tag="sqtot")
                        nc.scalar.activation(
                            out=sq_scr[:],
                            in_=scaledE[:, j * F * E : (j + 1) * F * E],
                            func=mybir.ActivationFunctionType.Square,
                            accum_out=sqtot[:],
                        )
                        f_scr = sc.tile([P, F], F32, tag="f_scr")
                        first = sc.tile([P, 1], F32, tag="first")
                        nc.scalar.activation(
                            out=f_scr[:],
                            in_=scaledF[:, j * F : (j + 1) * F],
                            func=mybir.ActivationFunctionType.Copy,
                            accum_out=first[:],
                        )

                        # transposes of e2 cols into scT (feature-major)
                        scT = scT_bufs[t % 2]
                        tp = ps1.tile([P, NCHUNK * P], F32, tag="tp")
                        for c in range(NCHUNK):
                            f0, nf = _chunk_fields(c)
                            in_c = scaledE[
                                :, (j * F + f0) * E : (j * F + f0 + nf) * E
                            ]
                            nc.tensor.transpose(
                                out=tp[: nf * E, c * P : (c + 1) * P],
                                in_=in_c,
                                identity=ident[:],
                            )
                        # copy PSUM -> SBUF (split so the ones-row is preserved);
                        # alternate engines across tiles to balance DVE/ACT load
                        lastK = (F - (NCHUNK - 1) * FPC) * E     # 112
                        if t % 2 == 0:
                            nc.vector.tensor_copy(
                                out=scT[:, : (NCHUNK - 1) * P],
                                in_=tp[:, : (NCHUNK - 1) * P],
                            )
                            nc.scalar.copy(
                                out=scT[:lastK, (NCHUNK - 1) * P :],
                                in_=tp[:lastK, (NCHUNK - 1) * P :],
                            )
                        else:
                            nc.scalar.copy(
                                out=scT[:, : (NCHUNK - 1) * P],
                                in_=tp[:, : (NCHUNK - 1) * P],
                            )
                            nc.vector.tensor_copy(
                                out=scT[:lastK, (NCHUNK - 1) * P :],
                                in_=tp[:lastK, (NCHUNK - 1) * P :],
                            )

                        # matmuls: out48 = scT.T @ [W1|S1]  (+ bias row on last chunk)
                        o48 = ps.tile([P, NR], F32, tag="o48")
                        for c in range(NCHUNK):
                            _, nf = _chunk_fields(c)
                            k = nf * E + (1 if c == NCHUNK - 1 else 0)  # +ones row
                            nc.tensor.matmul(
                                out=o48[:],
                                lhsT=scT[:k, c * P : c * P + P],
                                rhs=r_sb[:k, c * NR : (c + 1) * NR],
                                start=(c == 0),
                                stop=(c == NCHUNK - 1),
                            )

                        # h1 relu (ACT), then transpose -> h1T
                        h1r = sc.tile([P, H1], F32, tag="h1r")
                        nc.scalar.activation(
                            out=h1r[:],
                            in_=o48[:, :H1],
                            func=mybir.ActivationFunctionType.Relu,
                        )
                        h1T = h1T_bufs[t % 2]
                        tp2 = ps1.tile([H1, P], F32, tag="tp2")
                        nc.tensor.transpose(out=tp2[:], in_=h1r[:], identity=ident[:])
                        nc.vector.tensor_copy(out=h1T[:H1, :], in_=tp2[:])

                        # h2 = relu(h1 @ W2 + b2), hsum = sum_j h2
                        h2p = ps1.tile([P, H2], F32, tag="h2p")
                        nc.tensor.matmul(
                            out=h2p[:], lhsT=h1T[:], rhs=w2_sb[:],
                            start=True, stop=True,
                        )
                        h2r = sc.tile([P, H2], F32, tag="h2r")
                        nc.scalar.activation(
                            out=h2r[:],
                            in_=h2p[:],
                            func=mybir.ActivationFunctionType.Relu,
                        )
                        hsum = sc.tile([P, 1], F32, tag="hsum")
                        nc.vector.reduce_sum(
                            out=hsum[:], in_=h2r[:], axis=mybir.AxisListType.X
                        )

                        # fm = 0.5 * (sum_e s_e^2 - sqtot)
                        # (tensor_tensor_reduce crashes HW -- use mul+reduce)
                        s_sb = sc.tile([P, E], F32, tag="s_sb")
                        nc.vector.tensor_copy(out=s_sb[:], in_=o48[:, H1:])
                        s_scr = sc.tile([P, E], F32, tag="s_scr")
                        nc.vector.tensor_tensor(
                            out=s_scr[:], in0=s_sb[:], in1=s_sb[:],
                            op=mybir.AluOpType.mult,
                        )
                        s2sum = sc.tile([P, 1], F32, tag="s2sum")
                        nc.vector.reduce_sum(
                            out=s2sum[:], in_=s_scr[:], axis=mybir.AxisListType.X
                        )
                        # total = 0.5*(s2sum - sqtot) + first + hsum + bias
                        t1 = sc.tile([P, 1], F32, tag="t1")
                        nc.vector.tensor_tensor(
                            out=t1[:], in0=s2sum[:], in1=sqtot[:],
                            op=mybir.AluOpType.subtract,
                        )
                        t2 = sc.tile([P, 1], F32, tag="t2")
                        nc.vector.scalar_tensor_tensor(
                            out=t2[:],
                            in0=t1[:],
                            scalar=0.5,
                            in1=first[:],
                            op0=mybir.AluOpType.mult,
                            op1=mybir.AluOpType.add,
                        )
                        t3 = sc.tile([P, 1], F32, tag="t3")
                        nc.vector.tensor_tensor(
                            out=t3[:], in0=t2[:], in1=hsum[:],
                            op=mybir.AluOpType.add,
                        )
                        nc.vector.tensor_tensor(
                            out=res[:, t : t + 1], in0=t3[:], in1=cbc_sb[:],
                            op=mybir.AluOpType.add,
                        )

            # write out: transpose res [P, nt] -> [nt, P] and DMA
            resT = ps1.tile([nt, P], F32, tag="resT")
            nc.tensor.transpose(out=resT[:], in_=res[:], identity=ident[:])
            resTs = cpool.tile([nt, P], F32)
            nc.vector.tensor_copy(out=resTs[:], in_=resT[:])
            nc.sync.dma_start(
                out=out.rearrange("(t p) -> t p", p=P), in_=resTs[:]
            )

    nc.compile()
    return nc


def host_prep(Xi, Xv, emb1, emb2, W1, b1, W2, b2, bias, v=V):
    """Build the DRAM-side arrays the kernel consumes."""
    f = emb2.shape[0]
    e = emb2.shape[2]
    tbl = np.empty((f * v, ROW), np.float32)
    tbl[:, :E] = np.asarray(emb2, np.float32).reshape(f * v, e)
    tbl[:, E] = np.asarray(emb1, np.float32).reshape(f * v)
    off = np.asarray(Xi[:, :, 0], np.int32) + (np.arange(f, dtype=np.int32) * v)[None, :]
    xv = np.ascontiguousarray(np.asarray(Xv, np.float32))

    s1 = np.tile(np.eye(E, dtype=np.float32), (f, 1))
    rmat = np.zeros((NCHUNK * P, H1 + E), np.float32)
    rmat[: f * e, :H1] = np.asarray(W1, np.float32)
    rmat[: f * e, H1:] = s1
    rmat[f * e, :H1] = np.asarray(b1, np.float32)      # bias row (row 624)
    w2b = np.concatenate(
        [np.asarray(W2, np.float32), np.asarray(b2, np.float32)[None, :]], axis=0
    )
    cbc = np.full((P, 1), np.float32(np.asarray(bias).reshape(-1)[0]), np.float32)
    return tbl, off, xv, rmat, w2b, cbc


_NC_CACHE = {}

# test harness hooks: set TRACE=True before calling kernel() to profile; the
# last BassKernelResults lands in LAST_RESULT.
TRACE = False
LAST_RESULT = None


def make_in_maps(Xi, Xv, emb1, emb2, W1, b1, W2, b2, bias):
    ns = Xi.shape[0] // NCORES
    tbl, off, xv, rmat, w2b, cbc = host_prep(
        Xi, Xv, emb1, emb2, W1, b1, W2, b2, bias
    )
    in_maps = []
    for c in range(NCORES):
        in_maps.append(
            {
                "tbl": tbl,
                "off": np.ascontiguousarray(off[c * ns : (c + 1) * ns]),
                "xv": np.ascontiguousarray(xv[c * ns : (c + 1) * ns]),
                "rmat": rmat,
                "w2b": w2b,
                "cbc": cbc,
            }
        )
    return in_maps


def kernel(Xi, Xv, emb1, emb2, W1, b1, W2, b2, bias):
    ns = Xi.shape[0] // NCORES
    group = int(os.environ.get("DEEPFM_GROUP", "2"))
    nq = int(os.environ.get("DEEPFM_NQ", "1"))
    key = (ns, V, group, NCORES, nq)
    if key not in _NC_CACHE:
        _NC_CACHE[key] = build_nc(ns, V, group, NCORES, nq)
    nc = _NC_CACHE[key]

    in_maps = make_in_maps(Xi, Xv, emb1, emb2, W1, b1, W2, b2, bias)
    res = run_bass_kernel_spmd(nc, in_maps, list(range(NCORES)), trace=TRACE)
    global LAST_RESULT
    LAST_RESULT = res
    out = np.concatenate([res.results[c]["out"] for c in range(NCORES)])
    return out.astype(np.float32)



# revision 2
# speedup vs baseline: 1.3568x; 1.3568x over previous
"""DeepFM Trainium2 Bass kernel, v2: multi-queue dma_gather embedding fetch.

Strategy: batch-parallel across 8 NeuronCores, embedding tables replicated.
The gather bottleneck is Q7 SWDGE descriptor generation (~8 ns/row on one
queue); InstDMAGatherAnt dispatches desc-gen to the Q7 core pair selected by
queue_num, so cycling queues 0-3 runs generation ~3x parallel (~2.7 ns/row).

Table layout: per field, vocab rows are packed 3-per-256B-group:
  group g = [e2(3g) e1(3g) | e2(3g+1) e1(3g+1) | e2(3g+2) e1(3g+2) | pad13]
dma_gather idx = Xi//3 - 16384 (signed int16; the ucode's address math is
unsigned-stride x SIGNED-idx, so a mid-table base covers all 33334 groups).
Each instruction's idx list gets a 16-entry zero tail so the ucode's
trailing-negative trim never drops real rows.

Row selection (Xi mod 3) folds into the Xv scale: host ships
mxv[s,f,k] = Xv[s,f] * (Xi[s,f]%3==k); the unselected rows scale to exact
0.0 and flow through FM-square/accums and the K-expanded (39*3*16) MLP
matmul as zeros.

Per core (ns=2048 samples): 4 stages x 4 tiles of 128 samples.
Per stage: 39 dma_gathers (one per field, 528 idxs, queue f%4) into a
[128, 39*5*64] dest; per tile: DVE scale+select, ACT square/first-order
accums, 15 PE chunk transposes + matmuls vs [W1|S1], small MLP + FM combine
as in v1.
"""

import os
from contextlib import nullcontext

import numpy as np

import concourse.bass as bass
import concourse.mybir as mybir
import concourse.tile as tile
from concourse import bacc
from concourse.bass_utils import run_bass_kernel_spmd
from concourse.masks import make_identity

F32 = mybir.dt.float32
I16 = mybir.dt.int16

F = 39
V = 100000
E = 16
B = 16384
H1 = 32
H2 = 32
NCORES = 8
P = 128

G3 = 3                        # vocab rows per 256B group
NG = 33334                    # groups per field (3*33334 = 100002, 2 phantom)
IBASE = 16384                 # idx base: signed idx = Xi//3 - IBASE
GELEM = 64                    # f32 per group (256B)
NIDX = 528                    # 512 real + 16 zero-tail idxs per gather
GCH = 5                       # ceil(528/128) dest chunks per field
NSTAGE = 4
TPS = 4                       # tiles per stage
KTOT = F * G3 * E             # 1872 matmul K rows
NCHUNK = 15                   # ceil(1873/128)
LASTK = KTOT - 14 * P         # 80 data rows in chunk 14
NR = H1 + E                   # 48 combined rhs cols


def build_nc(ns, v, group, num_devices, nq=1, repeat=1):
    """Per-core Bass program. Signature kept compatible with v1 test.py;
    group/nq are ignored (queues are always 0-3)."""
    assert ns == NSTAGE * TPS * P
    nt = ns // P

    nc = bacc.Bacc(
        "TRN2", target_bir_lowering=False, debug=False, num_devices=num_devices,
        num_swdge_queues=4,
    )

    tbl = nc.dram_tensor("tbl", [F * NG, GELEM], F32, kind="ExternalInput").ap()
    idxa = nc.dram_tensor("idxa", [NSTAGE * P, F * (NIDX // 16)], I16,
                          kind="ExternalInput").ap()
    mxva = nc.dram_tensor("mxva", [NSTAGE * P, TPS * F * G3], F32,
                          kind="ExternalInput").ap()
    rmat = nc.dram_tensor("rmat", [NCHUNK * P, NR], F32, kind="ExternalInput").ap()
    w2b = nc.dram_tensor("w2b", [H1 + 1, H2], F32, kind="ExternalInput").ap()
    cbc = nc.dram_tensor("cbc", [P, 1], F32, kind="ExternalInput").ap()
    out = nc.dram_tensor("out", [ns], F32, kind="ExternalOutput").ap()

    ICOL = NIDX // 16          # 33 idx cols per field

    with tile.TileContext(nc) as tc:
        with (
            tc.tile_pool(name="const", bufs=1) as cpool,
            tc.tile_pool(name="sb", bufs=2) as sb,      # per-stage tiles
            tc.tile_pool(name="sc", bufs=2) as sc,      # per-tile scratch
            tc.tile_pool(name="ps", bufs=2, space="PSUM") as ps,    # o48
            tc.tile_pool(name="ps1", bufs=1, space="PSUM") as ps1,  # tp tp2 h2p resT
        ):
            ident = cpool.tile([P, P], F32)
            make_identity(nc, ident[:])

            r_sb = cpool.tile([P, NCHUNK * NR], F32)
            nc.sync.dma_start(
                out=r_sb[:].rearrange("p (c r) -> p c r", c=NCHUNK),
                in_=rmat.rearrange("(c p) r -> p c r", p=P),
            )
            w2_sb = cpool.tile([H1 + 1, H2], F32)
            nc.sync.dma_start(out=w2_sb[:], in_=w2b)
            cbc_sb = cpool.tile([P, 1], F32)
            nc.sync.dma_start(out=cbc_sb[:], in_=cbc)

            # scT double buffers; ones-row for the bias trick lives at row 80
            # of chunk 14 (engines can memset from partition 64), rows 64..79
            # are overwritten by every tile's PSUM copy.
            scT_bufs = []
            for i in range(2):
                t = cpool.tile([P, NCHUNK * P], F32, tag=f"scT{i}")
                nc.vector.memset(t[64:P, 14 * P : 15 * P], 1.0)
                scT_bufs.append(t)
            h1T_bufs = []
            for i in range(2):
                t = cpool.tile([H1 + 1, P], F32, tag=f"h1T{i}")
                nc.vector.memset(t[H1 : H1 + 1, :], 1.0)
                h1T_bufs.append(t)

            res = cpool.tile([P, nt], F32)

            loop_cm = tc.For_i(0, repeat, 1) if repeat > 1 else nullcontext()
            with loop_cm:
                for st in range(NSTAGE):
                    idxt = sb.tile([P, F * ICOL], I16, tag="idxt")
                    nc.sync.dma_start(
                        out=idxt[:], in_=idxa[st * P : (st + 1) * P, :]
                    )
                    mxvt = sb.tile([P, TPS * F * G3], F32, tag="mxvt")
                    nc.sync.dma_start(
                        out=mxvt[:], in_=mxva[st * P : (st + 1) * P, :]
                    )
                    dest = sb.tile([P, F * GCH * GELEM], F32, tag="dest")
                    for f in range(F):
                        nc.gpsimd.dma_gather(
                            out_ap=dest[:, f * GCH * GELEM : (f + 1) * GCH * GELEM]
                            .rearrange("p (c e) -> p c e", e=GELEM),
                            in_ap=tbl[f * NG + IBASE : (f + 1) * NG, :],
                            idxs_ap=idxt[:, f * ICOL : (f + 1) * ICOL],
                            num_idxs=NIDX,
                            num_idxs_reg=NIDX,
                            elem_size=GELEM,
                            single_packet=False,
                            queue_num=f % 4,
                        )

                    dest4 = dest[:].rearrange(
                        "p (f c e) -> p f c e", f=F, c=GCH
                    )
                    mxv4 = mxvt[:].rearrange(
                        "p (c f k) -> p c f k", c=TPS, f=F
                    )

                    for j in range(TPS):
                        t = st * TPS + j

                        # e2 part: [p, f, k(3), e(16)] scaled by masked Xv
                        e2in = dest4[:, :, j, 0 : G3 * (E + 1)].rearrange(
                            "p f (k e) -> p f k e", k=G3
                        )
                        se2 = sc.tile([P, KTOT], F32, tag="se2")
                        nc.vector.tensor_tensor(
                            out=se2[:].rearrange("p (f k e) -> p f k e", k=G3, e=E),
                            in0=e2in[:, :, :, 0:E],
                            in1=mxv4[:, j].to_broadcast([P, F, G3, E]),
                            op=mybir.AluOpType.mult,
                        )
                        se1 = sc.tile([P, F * G3], F32, tag="se1")
                        nc.vector.tensor_tensor(
                            out=se1[:].rearrange("p (f k) -> p f k", k=G3),
                            in0=e2in[:, :, :, E].squeeze(),
                            in1=mxv4[:, j],
                            op=mybir.AluOpType.mult,
                        )

                        # ACT: sum of squares of e2, first-order sum
                        sq_scr = sc.tile([P, KTOT], F32, tag="sq_scr")
                        sqtot = sc.tile([P, 1], F32, tag="sqtot")
                        nc.scalar.activation(
                            out=sq_scr[:],
                            in_=se2[:],
                            func=mybir.ActivationFunctionType.Square,
                            accum_out=sqtot[:],
                        )
                        f_scr = sc.tile([P, F * G3], F32, tag="f_scr")
                        first = sc.tile([P, 1], F32, tag="first")
                        nc.scalar.activation(
                            out=f_scr[:],
                            in_=se1[:],
                            func=mybir.ActivationFunctionType.Copy,
                            accum_out=first[:],
                        )

                        # transposes of se2 into scT, two PSUM rounds
                        scT = scT_bufs[t % 2]
                        tp = ps1.tile([P, 8 * P], F32, tag="tp")
                        for c in range(8):
                            nc.tensor.transpose(
                                out=tp[:, c * P : (c + 1) * P],
                                in_=se2[:, c * P : (c + 1) * P],
                                identity=ident[:],
                            )
                        if t % 2 == 0:
                            nc.vector.tensor_copy(
                                out=scT[:, : 8 * P], in_=tp[:]
                            )
                        else:
                            nc.scalar.copy(out=scT[:, : 8 * P], in_=tp[:])
                        tp2r = ps1.tile([P, 8 * P], F32, tag="tp")
                        for c in range(8, 14):
                            nc.tensor.transpose(
                                out=tp2r[:, (c - 8) * P : (c - 7) * P],
                                in_=se2[:, c * P : (c + 1) * P],
                                identity=ident[:],
                            )
                        nc.tensor.transpose(
                            out=tp2r[:LASTK, 6 * P : 7 * P],
                            in_=se2[:, 14 * P : 14 * P + LASTK],
                            identity=ident[:],
                        )
                        if t % 2 == 0:
                            nc.scalar.copy(
                                out=scT[:, 8 * P : 14 * P], in_=tp2r[:, : 6 * P]
                            )
                            nc.vector.tensor_copy(
                                out=scT[:LASTK, 14 * P : 15 * P],
                                in_=tp2r[:LASTK, 6 * P : 7 * P],
                            )
                        else:
                            nc.vector.tensor_copy(
                                out=scT[:, 8 * P : 14 * P], in_=tp2r[:, : 6 * P]
                            )
                            nc.scalar.copy(
                                out=scT[:LASTK, 14 * P : 15 * P],
                                in_=tp2r[:LASTK, 6 * P : 7 * P],
                            )

                        # matmuls: o48 = scT.T @ [W1|S1] (+ bias row, chunk 14)
                        o48 = ps.tile([P, NR], F32, tag="o48")
                        for c in range(NCHUNK):
                            k = P if c < 14 else LASTK + 1
                            nc.tensor.matmul(
                                out=o48[:],
                                lhsT=scT[:k, c * P : c * P + P],
                                rhs=r_sb[:k, c * NR : (c + 1) * NR],
                                start=(c == 0),
                                stop=(c == NCHUNK - 1),
                            )

                        # h1 relu, transpose, h2 = relu(h1 @ W2 + b2), row-sum
                        h1r = sc.tile([P, H1], F32, tag="h1r")
                        nc.scalar.activation(
                            out=h1r[:],
                            in_=o48[:, :H1],
                            func=mybir.ActivationFunctionType.Relu,
                        )
                        h1T = h1T_bufs[t % 2]
                        tp2 = ps1.tile([H1, P], F32, tag="tp2")
                        nc.tensor.transpose(out=tp2[:], in_=h1r[:], identity=ident[:])
                        nc.vector.tensor_copy(out=h1T[:H1, :], in_=tp2[:])
                        h2p = ps1.tile([P, H2], F32, tag="h2p")
                        nc.tensor.matmul(
                            out=h2p[:], lhsT=h1T[:], rhs=w2_sb[:],
                            start=True, stop=True,
                        )
                        h2r = sc.tile([P, H2], F32, tag="h2r")
                        nc.scalar.activation(
                            out=h2r[:],
                            in_=h2p[:],
                            func=mybir.ActivationFunctionType.Relu,
                        )
                        hsum = sc.tile([P, 1], F32, tag="hsum")
                        nc.vector.reduce_sum(
                            out=hsum[:], in_=h2r[:], axis=mybir.AxisListType.X
                        )

                        # fm = 0.5 * (sum_e s_e^2 - sqtot); total
                        s_sb = sc.tile([P, E], F32, tag="s_sb")
                        nc.vector.tensor_copy(out=s_sb[:], in_=o48[:, H1:])
                        s_scr = sc.tile([P, E], F32, tag="s_scr")
                        nc.vector.tensor_tensor(
                            out=s_scr[:], in0=s_sb[:], in1=s_sb[:],
                            op=mybir.AluOpType.mult,
                        )
                        s2sum = sc.tile([P, 1], F32, tag="s2sum")
                        nc.vector.reduce_sum(
                            out=s2sum[:], in_=s_scr[:], axis=mybir.AxisListType.X
                        )
                        t1 = sc.tile([P, 1], F32, tag="t1")
                        nc.vector.tensor_tensor(
                            out=t1[:], in0=s2sum[:], in1=sqtot[:],
                            op=mybir.AluOpType.subtract,
                        )
                        t2 = sc.tile([P, 1], F32, tag="t2")
                        nc.vector.scalar_tensor_tensor(
                            out=t2[:],
                            in0=t1[:],
                            scalar=0.5,
                            in1=first[:],
                            op0=mybir.AluOpType.mult,
                            op1=mybir.AluOpType.add,
                        )
                        t3 = sc.tile([P, 1], F32, tag="t3")
                        nc.vector.tensor_tensor(
                            out=t3[:], in0=t2[:], in1=hsum[:],
                            op=mybir.AluOpType.add,
                        )
                        nc.vector.tensor_tensor(
                            out=res[:, t : t + 1], in0=t3[:], in1=cbc_sb[:],
                            op=mybir.AluOpType.add,
                        )

            resT = ps1.tile([nt, P], F32, tag="resT")
            nc.tensor.transpose(out=resT[:], in_=res[:], identity=ident[:])
            resTs = cpool.tile([nt, P], F32)
            nc.vector.tensor_copy(out=resTs[:], in_=resT[:])
            nc.sync.dma_start(
                out=out.rearrange("(t p) -> t p", p=P), in_=resTs[:]
            )

    nc.compile()
    return nc


def host_prep(Xi, Xv, emb1, emb2, W1, b1, W2, b2, bias, v=V):
    """Build the DRAM-side arrays the kernel consumes."""
    f = emb2.shape[0]
    e = emb2.shape[2]
    e2 = np.asarray(emb2, np.float32)
    e1 = np.asarray(emb1, np.float32).reshape(f, v)
    vpad = NG * G3
    v3 = np.zeros((f, vpad, e + 1), np.float32)
    v3[:, :v, :e] = e2
    v3[:, :v, e] = e1
    tbl = np.zeros((f * NG, GELEM), np.float32)
    tbl[:, : G3 * (e + 1)] = v3.reshape(f, NG, G3 * (e + 1)).reshape(
        f * NG, G3 * (e + 1)
    )

    xi = np.asarray(Xi[:, :, 0], np.int64)          # [B, F]
    xv = np.asarray(Xv, np.float32)                 # [B, F]
    idx16 = (xi // G3 - IBASE).astype(np.int16)     # [B, F]
    sel = (xi % G3).astype(np.int64)                # [B, F]

    rmat = np.zeros((NCHUNK * P, NR), np.float32)
    W1r = np.tile(
        np.asarray(W1, np.float32).reshape(f, 1, e, H1), (1, G3, 1, 1)
    ).reshape(KTOT, H1)
    rmat[:KTOT, :H1] = W1r
    rmat[:KTOT, H1:] = np.tile(np.eye(e, dtype=np.float32), (f * G3, 1))
    rmat[KTOT, :H1] = np.asarray(b1, np.float32)

    w2b = np.concatenate(
        [np.asarray(W2, np.float32), np.asarray(b2, np.float32)[None, :]], axis=0
    )
    cbc = np.full((P, 1), np.float32(np.asarray(bias).reshape(-1)[0]), np.float32)
    return tbl, idx16, sel, xv, rmat, w2b, cbc


def per_core_inputs(idx16, sel, xv, c, ns):
    """idxa [NSTAGE*P, F*33] i16 and mxva [NSTAGE*P, TPS*F*G3] f32 for core c."""
    lo = c * ns
    blk_i = idx16[lo : lo + ns]                    # [2048, F]
    blk_s = sel[lo : lo + ns]
    blk_x = xv[lo : lo + ns]
    ICOL = NIDX // 16

    idxa = np.zeros((NSTAGE, P, F, ICOL), np.int16)
    mxva = np.zeros((NSTAGE, P, TPS, F, G3), np.float32)
    spst = TPS * P                                  # samples per stage
    for st in range(NSTAGE):
        bi = blk_i[st * spst : (st + 1) * spst]     # [512, F]
        flat = np.concatenate([bi, np.zeros((NIDX - spst, F), np.int16)], axis=0)
        w = flat.reshape(ICOL, 16, F).transpose(1, 0, 2)   # [16, ICOL, F]
        idxa[st] = np.tile(w, (8, 1, 1)).transpose(0, 2, 1)

        bs = blk_s[st * spst : (st + 1) * spst].reshape(TPS, P, F)
        bx = blk_x[st * spst : (st + 1) * spst].reshape(TPS, P, F)
        m = np.zeros((TPS, P, F, G3), np.float32)
        cidx, pidx, fidx = np.meshgrid(
            np.arange(TPS), np.arange(P), np.arange(F), indexing="ij"
        )
        m[cidx, pidx, fidx, bs] = bx
        mxva[st] = m.transpose(1, 0, 2, 3)
    return (
        idxa.reshape(NSTAGE * P, F * ICOL),
        mxva.reshape(NSTAGE * P, TPS * F * G3),
    )


_NC_CACHE = {}

TRACE = False
LAST_RESULT = None


def make_in_maps(Xi, Xv, emb1, emb2, W1, b1, W2, b2, bias):
    ns = Xi.shape[0] // NCORES
    tbl, idx16, sel, xv, rmat, w2b, cbc = host_prep(
        Xi, Xv, emb1, emb2, W1, b1, W2, b2, bias
    )
    in_maps = []
    for c in range(NCORES):
        idxa, mxva = per_core_inputs(idx16, sel, xv, c, ns)
        in_maps.append(
            {
                "tbl": tbl,
                "idxa": idxa,
                "mxva": mxva,
                "rmat": rmat,
                "w2b": w2b,
                "cbc": cbc,
            }
        )
    return in_maps


def kernel(Xi, Xv, emb1, emb2, W1, b1, W2, b2, bias):
    ns = Xi.shape[0] // NCORES
    group = int(os.environ.get("DEEPFM_GROUP", "2"))
    nq = int(os.environ.get("DEEPFM_NQ", "1"))
    key = (ns, V, group, NCORES, nq)
    if key not in _NC_CACHE:
        _NC_CACHE[key] = build_nc(ns, V, group, NCORES, nq)
    nc = _NC_CACHE[key]

    in_maps = make_in_maps(Xi, Xv, emb1, emb2, W1, b1, W2, b2, bias)
    res = run_bass_kernel_spmd(nc, in_maps, list(range(NCORES)), trace=TRACE)
    global LAST_RESULT
    LAST_RESULT = res
    out = np.concatenate([res.results[c]["out"] for c in range(NCORES)])
    return out.astype(np.float32)
